# revision 1
# baseline (speedup 1.0000x reference)
"""Trainium2 Bass kernel for nn_CustomRelation (sparse_attention).

Per batch b:
    qkw = hidden @ W + bias            # [S, 128] = [q(64) | k(64)]
    RoPE(qkw) (interleaved pairs)
    logits[r] = q[i0[r]].k[i2[r]] + q[i1[r]].k[i3[r]]
    out = (logits + (1 - mask) * -1e12) / 8

Distribution: data-parallel over batch, 2 batches per NeuronCore x 8 cores.

Per-core dataflow (feature-major, SBUF-resident tables):
  - X tiles cast f32->bf16, transposed by DMA-xbar (2-byte transpose) into
    XT [128h, 512t] bf16; qkw^T [128f, 512t] = sum_k W_k^T @ XT_k on PE.
  - rot(qkw) via signed pair-swap permutation matmul.
  - sin/cos from position_ids on device: pg = pos*invf/(2pi) (K=1 matmul),
    f = pg - round(pg) (magic-number round), sin = Sin(2pi f),
    cos = Sin(-2pi|f| + pi/2) (ACT Sin domain is [-pi, pi]).
  - rope = qkw*cos + rot*sin accumulated into a per-batch feature-major
    SBUF table [128, 2048]; a half-swapped copy [k|q] is made via a
    permutation matmul (aligns k features with q partitions for the dots).
  - ap_gather (GPSIMD ucode) gathers tokens along the free dim entirely
    in SBUF; dot products via elementwise mul + ones-vector matmul
    (partition reduction on PE); mask+scale on [1, 2048]; store.
"""

import numpy as np

B, S, H, D = 16, 2048, 1024, 64
NCORES = 8
BC = B // NCORES            # batches per core
T = BC * S                  # tokens per core
DH = 2 * D                  # projected features (q|k)
MT = 512                    # macro-tile tokens
NM = T // MT                # macro-tiles per core
NG = MT // 128              # 128-token groups per macro-tile
KH = H // 128               # contraction chunks
MAGIC = 1.5 * 2.0 ** 23
TWO_PI = float(2.0 * np.pi)
HALF_PI = float(0.5 * np.pi)

_CACHE = {}


def _build_nc():
    import concourse.bass as bass
    import concourse.tile as tile
    from concourse import bacc, mybir, library_config

    f32 = mybir.dt.float32
    bf16 = mybir.dt.bfloat16
    i32 = mybir.dt.int32
    i16 = mybir.dt.int16
    Alu = mybir.AluOpType
    Act = mybir.ActivationFunctionType

    nc = bacc.Bacc("TRN2", target_bir_lowering=False, debug=False,
                   num_devices=NCORES)

    x = nc.dram_tensor("x", [T, H], f32, kind="ExternalInput")
    w = nc.dram_tensor("w", [H, DH], f32, kind="ExternalInput")
    bvec = nc.dram_tensor("bvec", [DH], f32, kind="ExternalInput")
    pos = nc.dram_tensor("pos", [BC, S], i32, kind="ExternalInput")
    idx16 = nc.dram_tensor("idx16", [BC, 2, 128, S // 16], i16,
                           kind="ExternalInput")
    msk = nc.dram_tensor("msk", [BC, S], f32, kind="ExternalInput")
    out = nc.dram_tensor("out", [BC, S], f32, kind="ExternalOutput")

    # --- constants baked into the NEFF ---
    invf = np.power(10000.0, -np.arange(D // 2, dtype=np.float64) / (D / 2.0))
    invf_rep = np.repeat(invf, 2)
    g_rep = np.concatenate([invf_rep, invf_rep]) / (2 * np.pi)
    g_rep = g_rep.reshape(1, DH).astype(np.float32)     # [1,128] lhsT (K=1)
    pswapT = np.zeros((DH, DH), dtype=np.float32)       # signed pair swap
    for j in range(D):
        pswapT[2 * j + 1, 2 * j] = -1.0                 # rot[2j]   = -x[2j+1]
        pswapT[2 * j, 2 * j + 1] = 1.0                  # rot[2j+1] =  x[2j]
    sqqT = np.zeros((DH, DH), dtype=np.float32)         # [q|k] -> [q|q]
    skkT = np.zeros((DH, DH), dtype=np.float32)         # [q|k] -> [k|k]
    for dd in range(DH):
        sqqT[dd % D, dd] = 1.0
        skkT[D + dd % D, dd] = 1.0
    grep_t = nc.inline_tensor(g_rep, "grep")
    pswap_t = nc.inline_tensor(pswapT, "pswapT")
    sqq_t = nc.inline_tensor(sqqT, "sqqT")
    skk_t = nc.inline_tensor(skkT, "skkT")
    ones_t = nc.inline_tensor(np.ones((DH, 1), dtype=np.float32), "ones")

    with tile.TileContext(nc) as tc, \
         tc.tile_pool(name="consts", bufs=1) as consts, \
         tc.tile_pool(name="xp", bufs=3) as xp, \
         tc.tile_pool(name="xt", bufs=2) as xtp, \
         tc.tile_pool(name="sb", bufs=2) as sbp, \
         tc.tile_pool(name="sc", bufs=2) as scp, \
         tc.tile_pool(name="fm", bufs=1) as fmp, \
         tc.tile_pool(name="gth", bufs=1) as gth, \
         tc.tile_pool(name="ps_qkw", bufs=2, space="PSUM") as ps_qkw, \
         tc.tile_pool(name="ps_rot", bufs=1, space="PSUM") as ps_rot, \
         tc.tile_pool(name="ps_pg", bufs=1, space="PSUM") as ps_pg, \
         tc.tile_pool(name="ps_fm2", bufs=1, space="PSUM") as ps_fm2, \
         tc.tile_pool(name="ps_lg", bufs=1, space="PSUM") as ps_lg:

        nc.gpsimd.load_library(library_config.ap_gather)

        # ---- constants / small inputs ----
        grep = consts.tile([1, DH], f32, tag="grep")
        nc.sync.dma_start(out=grep[:], in_=grep_t.ap())
        pswap = consts.tile([DH, DH], f32, tag="pswap")
        nc.sync.dma_start(out=pswap[:], in_=pswap_t.ap())
        sqq = consts.tile([DH, DH], f32, tag="sqq")
        nc.sync.dma_start(out=sqq[:], in_=sqq_t.ap())
        skk = consts.tile([DH, DH], f32, tag="skk")
        nc.sync.dma_start(out=skk[:], in_=skk_t.ap())
        ones = consts.tile([DH, 1], f32, tag="ones")
        nc.sync.dma_start(out=ones[:], in_=ones_t.ap())
        bcol = consts.tile([DH, 1], f32, tag="bcol")
        nc.sync.dma_start(out=bcol[:], in_=bvec.ap().rearrange("(p o) -> p o", o=1))
        wsb = []
        for k in range(KH):
            wf = consts.tile([128, DH], f32, tag=f"wf{k}")
            nc.sync.dma_start(out=wf[:], in_=w[128 * k:128 * (k + 1), :])
            wb = consts.tile([128, DH], bf16, tag=f"wb{k}")
            nc.vector.tensor_copy(out=wb[:], in_=wf[:])
            wsb.append(wb)
        pos_fb = []
        for b in range(BC):
            pib = consts.tile([1, S], i32, tag=f"pos_i{b}")
            nc.sync.dma_start(out=pib[:], in_=pos[b:b + 1, :])
            pfb = consts.tile([1, S], f32, tag=f"pos_f{b}")
            nc.vector.tensor_copy(out=pfb[:], in_=pib[:])
            pos_fb.append(pfb)
        zerob = consts.tile([DH, 1], f32, tag="zerob")
        nc.vector.memset(zerob[:], 0.0)
        pihalf = consts.tile([DH, 1], f32, tag="pihalf")
        nc.vector.memset(pihalf[:], HALF_PI)

        # per-batch feature-major tables (SBUF-resident)
        fqq = [fmp.tile([DH, S], f32, name=f"fqq{b}", tag=f"fqq{b}")
               for b in range(BC)]
        fkk = [fmp.tile([DH, S], f32, name=f"fkk{b}", tag=f"fkk{b}")
               for b in range(BC)]

        # ---- main pipeline over macro-tiles ----
        for m in range(NM):
            bi = m // (NM // BC)
            t0 = m * MT
            tl0 = t0 - bi * S

            # load + cast + DMA-transpose into XT bf16 [128h, k, 512t]
            xtall = xtp.tile([128, KH, MT], bf16, name=f"xta_{m}", tag="xta")
            for g in range(NG):
                xg = xp.tile([128, H], f32, tag="x")
                nc.sync.dma_start(
                    out=xg[:], in_=x[t0 + 128 * g:t0 + 128 * (g + 1), :])
                xb = xp.tile([128, H], bf16, tag="xb")
                if g % 2 == 0:
                    nc.vector.tensor_copy(out=xb[:], in_=xg[:])
                else:
                    nc.scalar.copy(out=xb[:], in_=xg[:])
                # one xbar transpose for all KH chunks: row 128k+p of x^T
                # lands at out[p, k, :]
                nc.sync.dma_start_transpose(
                    out=xtall[:, :, 128 * g:128 * (g + 1)], in_=xb[:])
            xts = [xtall[:, k, :] for k in range(KH)]

            # qkw^T [128f, 512t] = sum_k W_k^T @ XT_k  (+bias)
            pq = ps_qkw.tile([DH, MT], f32, tag="qkw")
            for k in range(KH):
                nc.tensor.matmul(out=pq[:], lhsT=wsb[k][:], rhs=xts[k],
                                 start=(k == 0), stop=(k == KH - 1))
            qkw = sbp.tile([DH, MT], f32, tag="qkw_sb")
            nc.vector.tensor_scalar(out=qkw[:], in0=pq[:], scalar1=bcol[:],
                                    scalar2=None, op0=Alu.add)

            # rot(qkw)
            pr = ps_rot.tile([DH, MT], f32, tag="rot")
            nc.tensor.matmul(out=pr[:], lhsT=pswap[:], rhs=qkw[:],
                             start=True, stop=True)

            # sin/cos
            pg = ps_pg.tile([DH, MT], f32, tag="pg")
            nc.tensor.matmul(out=pg[:], lhsT=grep[:],
                             rhs=pos_fb[bi][:, tl0:tl0 + MT],
                             start=True, stop=True)
            tmag = sbp.tile([DH, MT], f32, tag="tmag")
            nc.scalar.activation(out=tmag[:], in_=pg[:], func=Act.Copy,
                                 bias=MAGIC)
            mrnd = sbp.tile([DH, MT], f32, tag="mrnd")
            nc.vector.tensor_scalar(out=mrnd[:], in0=tmag[:], scalar1=-MAGIC,
                                    scalar2=None, op0=Alu.add)
            frac = sbp.tile([DH, MT], f32, tag="frac")
            nc.vector.tensor_tensor(out=frac[:], in0=pg[:], in1=mrnd[:],
                                    op=Alu.subtract)
            sint = scp.tile([DH, MT], f32, tag="sin")
            nc.scalar.activation(out=sint[:], in_=frac[:], func=Act.Sin,
                                 scale=TWO_PI, bias=zerob[:])
            afrac = sbp.tile([DH, MT], f32, tag="afrac")
            nc.scalar.activation(out=afrac[:], in_=frac[:], func=Act.Abs,
                                 bias=zerob[:])
            cost = scp.tile([DH, MT], f32, tag="cos")
            nc.scalar.activation(out=cost[:], in_=afrac[:], func=Act.Sin,
                                 scale=-TWO_PI, bias=pihalf[:])

            # rope = qkw*cos + rot*sin  -> feature-major table slice
            t1 = sbp.tile([DH, MT], f32, tag="t1")
            nc.vector.tensor_tensor(out=t1[:], in0=qkw[:], in1=cost[:],
                                    op=Alu.mult)
            t2 = sbp.tile([DH, MT], f32, tag="t2")
            nc.vector.tensor_tensor(out=t2[:], in0=pr[:], in1=sint[:],
                                    op=Alu.mult)
            rp = sbp.tile([DH, MT], f32, tag="rope")
            nc.vector.tensor_tensor(out=rp[:], in0=t1[:], in1=t2[:],
                                    op=Alu.add)

            # [q|q] and [k|k] table slices via selector matmuls
            pf2 = ps_fm2.tile([DH, MT], f32, tag="fm2")
            nc.tensor.matmul(out=pf2[:], lhsT=sqq[:], rhs=rp[:],
                             start=True, stop=True)
            nc.scalar.copy(out=fqq[bi][:, tl0:tl0 + MT], in_=pf2[:])
            pf3 = ps_fm2.tile([DH, MT], f32, tag="fm3")
            nc.tensor.matmul(out=pf3[:], lhsT=skk[:], rhs=rp[:],
                             start=True, stop=True)
            nc.scalar.copy(out=fkk[bi][:, tl0:tl0 + MT], in_=pf3[:])

            # ---- per-batch tail: SBUF gathers + dots ----
            if m % (NM // BC) == (NM // BC) - 1:
                idxs = gth.tile([128, 2 * (S // 16)], i16, tag="idxs")
                nc.sync.dma_start(
                    out=idxs[:].rearrange("p (g s) -> p g s", g=2),
                    in_=idx16[bi].rearrange("g p s -> p g s"))
                msk1 = gth.tile([1, S], f32, tag="msk1")
                nc.sync.dma_start(out=msk1[:], in_=msk[bi:bi + 1, :])

                def gather(dst_ap, src_ap, idx_ap):
                    nc.gpsimd.ap_gather(
                        out_ap=dst_ap.rearrange("c (n d) -> c n d", d=1),
                        in_ap=src_ap.rearrange("c (n d) -> c n d", d=1),
                        idxs_ap=idx_ap,
                        channels=DH, num_elems=S, d=1, num_idxs=S)

                SW = S // 16
                ga = gth.tile([DH, S], f32, tag="ga")   # [q[i0] | q[i1]]
                gather(ga[:], fqq[bi][:], idxs[:, 0:SW])
                gb = gth.tile([DH, S], f32, tag="gb")   # [k[i2] | k[i3]]
                gather(gb[:], fkk[bi][:], idxs[:, SW:2 * SW])

                prod = gth.tile([DH, S], f32, tag="prod")
                nc.vector.tensor_tensor(out=prod[:], in0=ga[:],
                                        in1=gb[:], op=Alu.mult)

                lgsb = gth.tile([1, S], f32, tag="lgsb")
                for c in range(S // MT):
                    pl = ps_lg.tile([1, MT], f32, tag="lg")
                    nc.tensor.matmul(out=pl[:], lhsT=ones[:],
                                     rhs=prod[:, MT * c:MT * (c + 1)],
                                     start=True, stop=True)
                    # fused /sqrt(D): logits * 0.125
                    nc.scalar.activation(out=lgsb[:, MT * c:MT * (c + 1)],
                                         in_=pl[:], func=Act.Copy, scale=0.125)
                nc.scalar.activation(out=msk1[:], in_=msk1[:], func=Act.Copy,
                                     scale=1.25e11, bias=-1.25e11)
                nc.vector.tensor_tensor(out=lgsb[:], in0=lgsb[:], in1=msk1[:],
                                        op=Alu.add)
                nc.sync.dma_start(out=out[bi:bi + 1, :], in_=lgsb[:])

    nc.compile()
    return nc


def _get_nc():
    if "nc" not in _CACHE:
        _CACHE["nc"] = _build_nc()
    return _CACHE["nc"]


def _shard(inputs):
    lhs = np.ascontiguousarray(np.asarray(inputs["last_hidden_state"], np.float32))
    w = np.ascontiguousarray(np.asarray(inputs["W"], np.float32))
    b = np.ascontiguousarray(np.asarray(inputs["b"], np.float32))
    pos = np.ascontiguousarray(np.asarray(inputs["position_ids"], np.int32))
    rel = np.ascontiguousarray(np.asarray(inputs["relations_idx"], np.int32))
    msk = np.ascontiguousarray(np.asarray(inputs["labels_mask"], np.float32))
    in_maps = []
    for c in range(NCORES):
        sl = slice(BC * c, BC * (c + 1))
        relc = rel[sl]
        # wrapped index layout for ap_gather: [type, 128, S//16] int16;
        # partition 16c+j holds indices i = s*16+j (replica per Q7 core)
        wrp = np.tile(relc.transpose(0, 2, 1)            # [BC, 4, S]
                      .reshape(BC, 4, S // 16, 16)       # [BC, 4, s, j]
                      .transpose(0, 1, 3, 2),            # [BC, 4, j, s]
                      (1, 1, 4, 1)).astype(np.int16)     # [BC, 4, 64, S//16]
        idx16c = np.ascontiguousarray(np.stack(
            [np.concatenate([wrp[:, 0], wrp[:, 1]], axis=1),    # [i0 | i1]
             np.concatenate([wrp[:, 2], wrp[:, 3]], axis=1)],   # [i2 | i3]
            axis=1))                                     # [BC, 2, 128, S//16]
        in_maps.append({
            "x": lhs[sl].reshape(T, H).copy(),
            "w": w,
            "bvec": b,
            "pos": pos[sl].copy(),
            "idx16": idx16c,
            "msk": msk[sl].copy(),
        })
    return in_maps


def kernel(**inputs):
    from concourse import bass_utils
    nc = _get_nc()
    in_maps = _shard(inputs)
    res = bass_utils.run_bass_kernel_spmd(
        nc, in_maps, core_ids=list(range(NCORES)))
    _CACHE["last_results"] = res
    outs = [res.results[c]["out"].reshape(BC, S) for c in range(NCORES)]
    return np.concatenate(outs, axis=0).astype(np.float32)



# revision 12
# speedup vs baseline: 1.4940x; 1.4940x over previous
"""Trainium2 Bass kernel for nn_CustomRelation (sparse_attention).

Per batch b:
    qkw = hidden @ W + bias            # [S, 128] = [q(64) | k(64)]
    RoPE(qkw) (interleaved pairs)
    logits[r] = q[i0[r]].k[i2[r]] + q[i1[r]].k[i3[r]]
    out = (logits + (1 - mask) * -1e12) / 8

Distribution: data-parallel over batch, 2 batches per NeuronCore x 8 cores.

Per-core dataflow (v2 — DMA-gather, host-prepped layouts):
  - Host pre-transposes x to bf16 x^T tiles [m, 128h, k, 512t] and
    precomputes feature-major sin/cos tables (bf16), so the device does
    no f32 x loads, no casts, no x transposes, and no sin/cos chain.
  - qkw^T [128f, 512t] = sum_k W_k^T @ XT_k on PE (bf16), +bias fused
    into the psum->sbuf bf16 cast on Scalar.
  - rot(qkw) via signed pair-swap permutation matmul (bf16).
  - rope = qkw*cos + rot*sin (DVE), output bf16 [128f, 512t].
  - One DMA-xbar transpose per tile packs rope token-major into a
    per-batch SBUF table: token t -> partition t%128, 256B row t//128.
  - 4x gpsimd.dma_gather (SWDGE, transpose mode, SBUF source) gather
    rows i0/i1/i2/i3 feature-major [128f, 2048r] in ~2-3us each
    (vs ~50us for the GPSIMD ap_gather ucode).
  - dots: prod[0:64] = G0.q * G2.k, prod[64:128] = G1.q * G3.k
    (partition-shifted DVE reads), ones-matmul partition reduction,
    mask+scale on [1, 2048]; store.
"""

import numpy as np

B, S, H, D = 16, 2048, 1024, 64
NCORES = 8
BC = B // NCORES            # batches per core
T = BC * S                  # tokens per core
DH = 2 * D                  # projected features (q|k)
MT = 512                    # macro-tile tokens
NM = T // MT                # macro-tiles per core
NMB = NM // BC              # macro-tiles per batch
KH = H // 128               # contraction chunks
CPB = S // 128              # 256B table rows per partition per batch

_CACHE = {}


def _build_nc():
    import concourse.bass as bass
    import concourse.tile as tile
    from concourse import bacc, mybir, library_config

    f32 = mybir.dt.float32
    bf16 = mybir.dt.bfloat16
    i16 = mybir.dt.int16
    Alu = mybir.AluOpType
    Act = mybir.ActivationFunctionType

    nc = bacc.Bacc("TRN2", target_bir_lowering=False, debug=False,
                   num_devices=NCORES)

    xt = nc.dram_tensor("xt", [NM, 128, KH, MT], bf16, kind="ExternalInput")
    wt = nc.dram_tensor("wt", [KH, 128, DH], bf16, kind="ExternalInput")
    bvec = nc.dram_tensor("bvec", [DH], f32, kind="ExternalInput")
    sct = nc.dram_tensor("sct", [2, DH, T], bf16, kind="ExternalInput")
    idx16 = nc.dram_tensor("idx16", [BC, 4, 128, S // 16], i16,
                           kind="ExternalInput")
    msk = nc.dram_tensor("msk", [BC, S], f32, kind="ExternalInput")
    out = nc.dram_tensor("out", [BC, S], f32, kind="ExternalOutput")

    # --- constants baked into the NEFF ---
    import ml_dtypes
    pswapT = np.zeros((DH, DH), dtype=np.float32)       # signed pair swap
    for j in range(D):
        pswapT[2 * j + 1, 2 * j] = -1.0                 # rot[2j]   = -x[2j+1]
        pswapT[2 * j, 2 * j + 1] = 1.0                  # rot[2j+1] =  x[2j]
    pswap_t = nc.inline_tensor(pswapT.astype(ml_dtypes.bfloat16), "pswapT")
    ones_t = nc.inline_tensor(
        np.ones((DH, 1), dtype=ml_dtypes.bfloat16), "ones")

    import os
    gmode = os.environ.get("KDBG_GATHER", "hbm_t")

    with tile.TileContext(nc) as tc, \
         tc.tile_pool(name="consts", bufs=1) as consts, \
         tc.tile_pool(name="xp", bufs=3) as xp, \
         tc.tile_pool(name="sc", bufs=3) as scp, \
         tc.tile_pool(name="sb", bufs=3) as sbp, \
         tc.tile_pool(name="fm", bufs=1) as fmp, \
         tc.tile_pool(name="gth", bufs=2) as gth, \
         tc.tile_pool(name="tblh", bufs=1, space="DRAM") as tblh, \
         tc.tile_pool(name="ps_qkw", bufs=2, space="PSUM") as ps_qkw, \
         tc.tile_pool(name="ps_rot", bufs=2, space="PSUM") as ps_rot, \
         tc.tile_pool(name="ps_lg", bufs=2, space="PSUM") as ps_lg:

        nc.gpsimd.load_library(library_config.mlp)

        # ---- constants / small inputs ----
        pswap = consts.tile([DH, DH], bf16, tag="pswap")
        nc.sync.dma_start(out=pswap[:], in_=pswap_t.ap())
        ones = consts.tile([DH, 1], bf16, tag="ones")
        nc.sync.dma_start(out=ones[:], in_=ones_t.ap())
        bcol = consts.tile([DH, 1], f32, tag="bcol")
        nc.sync.dma_start(out=bcol[:], in_=bvec.ap().rearrange("(p o) -> p o", o=1))
        wsb = consts.tile([128, KH, DH], bf16, tag="wsb")
        nc.sync.dma_start(out=wsb[:], in_=wt.ap().rearrange("k p f -> p k f"))

        # per-batch token-major rope tables: token t -> partition t%128,
        # 256B row t//128 (dma_gather sbuf layout: tpr=128, 256B/rank).
        # tA rows = [q|q], tB rows = [k|k] so the gathered dot operands
        # sit on equal partition bases (BIR requires it for TensorTensor).
        tA = [fmp.tile([128, CPB, DH], bf16, name=f"tA{b}", tag=f"tA{b}")
              for b in range(BC)]
        tB = [fmp.tile([128, CPB, DH], bf16, name=f"tB{b}", tag=f"tB{b}")
              for b in range(BC)]
        # HBM copies of the tables (token-major rows) for HBM-source gather
        tAh = [tblh.tile([S, DH], bf16, name=f"tAh{b}", tag=f"tAh{b}")
               for b in range(BC)]
        tBh = [tblh.tile([S, DH], bf16, name=f"tBh{b}", tag=f"tBh{b}")
               for b in range(BC)]

        # ---- main pipeline over macro-tiles ----
        for m in range(NM):
            bi = m // NMB
            mb = m % NMB
            t0 = m * MT

            xtile = xp.tile([128, KH, MT], bf16, tag="x")
            nc.sync.dma_start(out=xtile[:], in_=xt[m])
            sins = scp.tile([DH, MT], bf16, tag="sin")
            nc.scalar.dma_start(out=sins[:], in_=sct[0, :, t0:t0 + MT])
            coss = scp.tile([DH, MT], bf16, tag="cos")
            nc.scalar.dma_start(out=coss[:], in_=sct[1, :, t0:t0 + MT])

            # qkw^T [128f, 512t] = sum_k W_k^T @ XT_k
            pq = ps_qkw.tile([DH, MT], f32, tag="qkw")
            for k in range(KH):
                nc.tensor.matmul(out=pq[:], lhsT=wsb[:, k, :],
                                 rhs=xtile[:, k, :],
                                 start=(k == 0), stop=(k == KH - 1))
            qkwb = sbp.tile([DH, MT], bf16, tag="qkwb")
            nc.scalar.activation(out=qkwb[:], in_=pq[:], func=Act.Identity,
                                 bias=bcol[:])

            # rot(qkw)
            pr = ps_rot.tile([DH, MT], f32, tag="rot")
            nc.tensor.matmul(out=pr[:], lhsT=pswap[:], rhs=qkwb[:],
                             start=True, stop=True)

            # rope = qkw*cos + rot*sin  -> bf16
            t1 = sbp.tile([DH, MT], f32, tag="t1")
            nc.vector.tensor_tensor(out=t1[:], in0=qkwb[:], in1=coss[:],
                                    op=Alu.mult)
            t2 = sbp.tile([DH, MT], f32, tag="t2")
            nc.vector.tensor_tensor(out=t2[:], in0=pr[:], in1=sins[:],
                                    op=Alu.mult)
            rpb = sbp.tile([DH, MT], bf16, tag="rpb")
            nc.vector.tensor_tensor(out=rpb[:], in0=t1[:], in1=t2[:],
                                    op=Alu.add)

            # pack token-major into the per-batch tables (xbar transpose):
            # token t=128c+p of this tile -> t{A,B}[p, 4*mb+c, :]
            c0 = 4 * mb
            nc.sync.dma_start_transpose(
                out=tA[bi][:, c0:c0 + 4, 0:D], in_=rpb[0:D, :])
            nc.sync.dma_start_transpose(
                out=tA[bi][:, c0:c0 + 4, D:DH], in_=rpb[0:D, :])
            nc.sync.dma_start_transpose(
                out=tB[bi][:, c0:c0 + 4, 0:D], in_=rpb[D:DH, :])
            nc.sync.dma_start_transpose(
                out=tB[bi][:, c0:c0 + 4, D:DH], in_=rpb[D:DH, :])

            # ---- per-batch tail: DMA gathers + dots ----
            if mb == NMB - 1:
                idxt = gth.tile([128, 4, S // 16], i16, tag="idxs")
                nc.gpsimd.dma_start(out=idxt[:],
                                    in_=idx16[bi].rearrange("g p s -> p g s"))
                msk1 = gth.tile([1, S], f32, tag="msk1")
                nc.gpsimd.dma_start(out=msk1[:], in_=msk[bi:bi + 1, :])

                if gmode.startswith("hbm"):
                    # flush token-major tables to HBM for the gather
                    nc.sync.dma_start(
                        out=tAh[bi][:].rearrange("(c p) f -> p c f", p=128),
                        in_=tA[bi][:])
                    nc.sync.dma_start(
                        out=tBh[bi][:].rearrange("(c p) f -> p c f", p=128),
                        in_=tB[bi][:])
                    srcs = [tAh[bi], tAh[bi], tBh[bi], tBh[bi]]
                else:
                    srcs = [tA[bi], tA[bi], tB[bi], tB[bi]]

                # split each gather into GC-index chunks: the SWDGE ring
                # holds 128 in-flight descriptors/queue; a 2048-idx gather
                # needs 130 (> ring) and wedges the device. 1024 -> 66.
                GC = 1024
                g4 = []
                for j, tbl in enumerate(srcs):
                    if gmode == "hbm_nt":
                        gj = gth.tile([128, CPB, DH], bf16, tag=f"g{j}")
                    else:
                        gj = gth.tile([128, 1, S], bf16, tag=f"g{j}")
                    for h in range(S // GC):
                        isl = idxt[:, j, GC // 16 * h:GC // 16 * (h + 1)]
                        if gmode == "none":
                            continue
                        elif gmode == "hbm_t":
                            nc.gpsimd.dma_gather(
                                out_ap=gj[:, :, GC * h:GC * (h + 1)],
                                in_ap=tbl[:], idxs_ap=isl,
                                num_idxs=GC, num_idxs_reg=GC, elem_size=DH,
                                transpose=True)
                        elif gmode == "hbm_nt":
                            nc.gpsimd.dma_gather(
                                out_ap=gj[:, GC // 128 * h:GC // 128 * (h + 1), :],
                                in_ap=tbl[:], idxs_ap=isl,
                                num_idxs=GC, num_idxs_reg=GC, elem_size=DH,
                                transpose=False)
                        else:  # sbuf
                            nc.gpsimd.dma_gather(
                                out_ap=gj[:, :, GC * h:GC * (h + 1)],
                                in_ap=tbl[:], idxs_ap=isl,
                                num_idxs=GC, num_idxs_reg=GC, elem_size=DH,
                                transpose=True,
                                sbuf_tokens_per_rank=128,
                                sbuf_free_dim_per_rank=2 * DH,
                                sbuf_free_dim_pad_per_rank=0,
                                sbuf_byte_offset=0)
                    if gmode == "none":
                        nc.vector.memset(gj[:], 0.0)
                    g4.append(gj)

                if gmode == "hbm_nt":
                    # wrapped layout: relation r at [r%128, r//128, f].
                    # padd[.,.,0:64] = q(i0)k(i2), [.,.,64:] = q(i1)k(i3)
                    padd = gth.tile([128, CPB, DH], bf16, tag="prod")
                    nc.vector.tensor_tensor(out=padd[:, :, 0:D],
                                            in0=g4[0][:, :, 0:D],
                                            in1=g4[2][:, :, D:DH], op=Alu.mult)
                    nc.vector.tensor_tensor(out=padd[:, :, D:DH],
                                            in0=g4[1][:, :, 0:D],
                                            in1=g4[3][:, :, D:DH], op=Alu.mult)
                    # xbar: prodb[f, w, p] with free pos (c*128+p) = r
                    prodb = gth.tile([128, CPB, 128], bf16, tag="prodt")
                    nc.sync.dma_start_transpose(out=prodb[:], in_=padd[:])
                    red = prodb[:].rearrange("f c p -> f (c p)")
                else:
                    # feature-major gathers: prod[0:64] = q(i0)*k(i2),
                    # prod[64:] = q(i1)*k(i3)
                    prodb = gth.tile([DH, S], bf16, tag="prod")
                    nc.vector.tensor_tensor(out=prodb[0:D, :],
                                            in0=g4[0][0:D, 0, :],
                                            in1=g4[2][0:D, 0, :], op=Alu.mult)
                    nc.vector.tensor_tensor(out=prodb[D:DH, :],
                                            in0=g4[1][D:DH, 0, :],
                                            in1=g4[3][D:DH, 0, :], op=Alu.mult)
                    red = prodb[:]

                nc.scalar.activation(out=msk1[:], in_=msk1[:], func=Act.Copy,
                                     scale=1.25e11, bias=-1.25e11)
                lgsb = gth.tile([1, S], f32, tag="lgsb")
                for c in range(S // MT):
                    pl = ps_lg.tile([1, MT], f32, tag="lg")
                    nc.tensor.matmul(out=pl[:], lhsT=ones[:],
                                     rhs=red[:, MT * c:MT * (c + 1)],
                                     start=True, stop=True)
                    # fused /sqrt(D): logits * 0.125
                    nc.scalar.activation(out=lgsb[:, MT * c:MT * (c + 1)],
                                         in_=pl[:], func=Act.Copy, scale=0.125)
                nc.vector.tensor_tensor(out=lgsb[:], in0=lgsb[:], in1=msk1[:],
                                        op=Alu.add)
                nc.scalar.dma_start(out=out[bi:bi + 1, :], in_=lgsb[:])

    nc.compile()
    return nc


def _get_nc():
    if "nc" not in _CACHE:
        _CACHE["nc"] = _build_nc()
    return _CACHE["nc"]


def _shard(inputs):
    import ml_dtypes
    bf16 = ml_dtypes.bfloat16
    lhs = np.asarray(inputs["last_hidden_state"], np.float32)
    w = np.asarray(inputs["W"], np.float32)
    b = np.ascontiguousarray(np.asarray(inputs["b"], np.float32))
    pos = np.asarray(inputs["position_ids"], np.int32)
    rel = np.asarray(inputs["relations_idx"], np.int32)
    msk = np.asarray(inputs["labels_mask"], np.float32)

    wtb = np.ascontiguousarray(w.astype(bf16).reshape(KH, 128, DH))
    invf = np.power(10000.0, -np.arange(D // 2, dtype=np.float64) / (D / 2.0))

    in_maps = []
    for c in range(NCORES):
        sl = slice(BC * c, BC * (c + 1))
        # x^T bf16 tiles: xt[m, p, k, j] = x[512m+j, 128k+p]
        xc = lhs[sl].reshape(T, H).astype(bf16)
        xtc = np.ascontiguousarray(
            xc.T.reshape(KH, 128, NM, MT).transpose(2, 1, 0, 3))
        # feature-major sin/cos (interleaved pairs, tiled q|k)
        ang = pos[sl].astype(np.float64)[:, :, None] * invf  # [BC,S,32]
        sin2 = np.repeat(np.sin(ang), 2, axis=-1)            # [BC,S,64]
        cos2 = np.repeat(np.cos(ang), 2, axis=-1)
        sfm = np.concatenate([sin2, sin2], axis=-1).transpose(2, 0, 1)
        cfm = np.concatenate([cos2, cos2], axis=-1).transpose(2, 0, 1)
        sctc = np.ascontiguousarray(
            np.stack([sfm.reshape(DH, T), cfm.reshape(DH, T)]).astype(bf16))
        # wrapped int16 indices for dma_gather: idx[i] at [i%16, i//16],
        # replicated to 128 partitions (8 Q7 cores)
        wrp = (rel[sl].transpose(0, 2, 1)            # [BC, 4, S]
               .reshape(BC, 4, S // 16, 16)          # [BC, 4, s, j]
               .transpose(0, 1, 3, 2))               # [BC, 4, 16, s]
        idx16c = np.ascontiguousarray(
            np.tile(wrp, (1, 1, 8, 1)).astype(np.int16))
        in_maps.append({
            "xt": xtc,
            "wt": wtb,
            "bvec": b,
            "sct": sctc,
            "idx16": idx16c,
            "msk": np.ascontiguousarray(msk[sl]),
        })
    return in_maps


def kernel(**inputs):
    from concourse import bass_utils
    nc = _get_nc()
    in_maps = _shard(inputs)
    res = bass_utils.run_bass_kernel_spmd(
        nc, in_maps, core_ids=list(range(NCORES)))
    _CACHE["last_results"] = res
    outs = [res.results[c]["out"].reshape(BC, S) for c in range(NCORES)]
    return np.concatenate(outs, axis=0).astype(np.float32)


# revision 13
# speedup vs baseline: 1.5285x; 1.0231x over previous
"""Trainium2 Bass kernel for nn_CustomRelation (sparse_attention).

Per batch b:
    qkw = hidden @ W + bias            # [S, 128] = [q(64) | k(64)]
    RoPE(qkw) (interleaved pairs)
    logits[r] = q[i0[r]].k[i2[r]] + q[i1[r]].k[i3[r]]
    out = (logits + (1 - mask) * -1e12) / 8

Distribution: data-parallel over batch, 2 batches per NeuronCore x 8 cores.

Per-core dataflow (v2 — DMA-gather, host-prepped layouts):
  - Host pre-transposes x to bf16 x^T tiles [m, 128h, k, 512t] and
    precomputes feature-major sin/cos tables (bf16), so the device does
    no f32 x loads, no casts, no x transposes, and no sin/cos chain.
  - qkw^T [128f, 512t] = sum_k W_k^T @ XT_k on PE (bf16), +bias fused
    into the psum->sbuf bf16 cast on Scalar.
  - rot(qkw) via signed pair-swap permutation matmul (bf16).
  - rope = qkw*cos + rot*sin (DVE), output bf16 [128f, 512t].
  - One DMA-xbar transpose per tile packs rope token-major into a
    per-batch SBUF table: token t -> partition t%128, 256B row t//128.
  - 4x gpsimd.dma_gather (SWDGE, transpose mode, SBUF source) gather
    rows i0/i1/i2/i3 feature-major [128f, 2048r] in ~2-3us each
    (vs ~50us for the GPSIMD ap_gather ucode).
  - dots: prod[0:64] = G0.q * G2.k, prod[64:128] = G1.q * G3.k
    (partition-shifted DVE reads), ones-matmul partition reduction,
    mask+scale on [1, 2048]; store.
"""

import numpy as np

B, S, H, D = 16, 2048, 1024, 64
NCORES = 8
BC = B // NCORES            # batches per core
T = BC * S                  # tokens per core
DH = 2 * D                  # projected features (q|k)
MT = 512                    # macro-tile tokens
NM = T // MT                # macro-tiles per core
NMB = NM // BC              # macro-tiles per batch
KH = H // 128               # contraction chunks
CPB = S // 128              # 256B table rows per partition per batch

_CACHE = {}


def _build_nc():
    import concourse.bass as bass
    import concourse.tile as tile
    from concourse import bacc, mybir, library_config

    f32 = mybir.dt.float32
    bf16 = mybir.dt.bfloat16
    i16 = mybir.dt.int16
    Alu = mybir.AluOpType
    Act = mybir.ActivationFunctionType

    nc = bacc.Bacc("TRN2", target_bir_lowering=False, debug=False,
                   num_devices=NCORES)

    xt = nc.dram_tensor("xt", [NM, 128, KH, MT], bf16, kind="ExternalInput")
    wt = nc.dram_tensor("wt", [KH, 128, DH], bf16, kind="ExternalInput")
    bvec = nc.dram_tensor("bvec", [DH], f32, kind="ExternalInput")
    sct = nc.dram_tensor("sct", [2, DH, T], bf16, kind="ExternalInput")
    idx16 = nc.dram_tensor("idx16", [BC, 4, 128, S // 16], i16,
                           kind="ExternalInput")
    msk = nc.dram_tensor("msk", [BC, S], f32, kind="ExternalInput")
    out = nc.dram_tensor("out", [BC, S], f32, kind="ExternalOutput")

    # --- constants baked into the NEFF ---
    import ml_dtypes
    pswapT = np.zeros((DH, DH), dtype=np.float32)       # signed pair swap
    for j in range(D):
        pswapT[2 * j + 1, 2 * j] = -1.0                 # rot[2j]   = -x[2j+1]
        pswapT[2 * j, 2 * j + 1] = 1.0                  # rot[2j+1] =  x[2j]
    pswap_t = nc.inline_tensor(pswapT.astype(ml_dtypes.bfloat16), "pswapT")
    ones_t = nc.inline_tensor(
        np.ones((DH, 1), dtype=ml_dtypes.bfloat16), "ones")

    import os
    # hbm_nt is the only dma_gather mode this runtime supports (SBUF-source
    # and transpose=True modes wedge the device; see transcript).
    gmode = os.environ.get("KDBG_GATHER", "hbm_nt")

    with tile.TileContext(nc) as tc, \
         tc.tile_pool(name="consts", bufs=1) as consts, \
         tc.tile_pool(name="xp", bufs=3) as xp, \
         tc.tile_pool(name="sc", bufs=3) as scp, \
         tc.tile_pool(name="sb", bufs=3) as sbp, \
         tc.tile_pool(name="fm", bufs=1) as fmp, \
         tc.tile_pool(name="gth", bufs=2) as gth, \
         tc.tile_pool(name="tblh", bufs=1, space="DRAM") as tblh, \
         tc.tile_pool(name="ps_qkw", bufs=2, space="PSUM") as ps_qkw, \
         tc.tile_pool(name="ps_rot", bufs=2, space="PSUM") as ps_rot, \
         tc.tile_pool(name="ps_lg", bufs=2, space="PSUM") as ps_lg:

        nc.gpsimd.load_library(library_config.mlp)

        # ---- constants / small inputs ----
        pswap = consts.tile([DH, DH], bf16, tag="pswap")
        nc.sync.dma_start(out=pswap[:], in_=pswap_t.ap())
        ones = consts.tile([DH, 1], bf16, tag="ones")
        nc.sync.dma_start(out=ones[:], in_=ones_t.ap())
        bcol = consts.tile([DH, 1], f32, tag="bcol")
        nc.sync.dma_start(out=bcol[:], in_=bvec.ap().rearrange("(p o) -> p o", o=1))
        wsb = consts.tile([128, KH, DH], bf16, tag="wsb")
        nc.sync.dma_start(out=wsb[:], in_=wt.ap().rearrange("k p f -> p k f"))

        # per-batch token-major rope tables: token t -> partition t%128,
        # 256B row t//128 (dma_gather sbuf layout: tpr=128, 256B/rank).
        # tA rows = [q|q], tB rows = [k|k] so the gathered dot operands
        # sit on equal partition bases (BIR requires it for TensorTensor).
        tA = [fmp.tile([128, CPB, DH], bf16, name=f"tA{b}", tag=f"tA{b}")
              for b in range(BC)]
        tB = [fmp.tile([128, CPB, DH], bf16, name=f"tB{b}", tag=f"tB{b}")
              for b in range(BC)]
        # HBM copies of the tables (token-major rows) for HBM-source gather
        tAh = [tblh.tile([S, DH], bf16, name=f"tAh{b}", tag=f"tAh{b}")
               for b in range(BC)]
        tBh = [tblh.tile([S, DH], bf16, name=f"tBh{b}", tag=f"tBh{b}")
               for b in range(BC)]

        # ---- main pipeline over macro-tiles ----
        for m in range(NM):
            bi = m // NMB
            mb = m % NMB
            t0 = m * MT

            xtile = xp.tile([128, KH, MT], bf16, tag="x")
            nc.sync.dma_start(out=xtile[:], in_=xt[m])
            sins = scp.tile([DH, MT], bf16, tag="sin")
            nc.scalar.dma_start(out=sins[:], in_=sct[0, :, t0:t0 + MT])
            coss = scp.tile([DH, MT], bf16, tag="cos")
            nc.scalar.dma_start(out=coss[:], in_=sct[1, :, t0:t0 + MT])

            # qkw^T [128f, 512t] = sum_k W_k^T @ XT_k
            pq = ps_qkw.tile([DH, MT], f32, tag="qkw")
            for k in range(KH):
                nc.tensor.matmul(out=pq[:], lhsT=wsb[:, k, :],
                                 rhs=xtile[:, k, :],
                                 start=(k == 0), stop=(k == KH - 1))
            qkwb = sbp.tile([DH, MT], bf16, tag="qkwb")
            nc.scalar.activation(out=qkwb[:], in_=pq[:], func=Act.Identity,
                                 bias=bcol[:])

            # rot(qkw)
            pr = ps_rot.tile([DH, MT], f32, tag="rot")
            nc.tensor.matmul(out=pr[:], lhsT=pswap[:], rhs=qkwb[:],
                             start=True, stop=True)

            # rope = qkw*cos + rot*sin  -> bf16
            t1 = sbp.tile([DH, MT], f32, tag="t1")
            nc.vector.tensor_tensor(out=t1[:], in0=qkwb[:], in1=coss[:],
                                    op=Alu.mult)
            t2 = sbp.tile([DH, MT], f32, tag="t2")
            nc.vector.tensor_tensor(out=t2[:], in0=pr[:], in1=sins[:],
                                    op=Alu.mult)
            rpb = sbp.tile([DH, MT], bf16, tag="rpb")
            nc.vector.tensor_tensor(out=rpb[:], in0=t1[:], in1=t2[:],
                                    op=Alu.add)

            # pack token-major into the per-batch tables (xbar transpose):
            # token t=128c+p of this tile -> t{A,B}[p, 4*mb+c, :]
            c0 = 4 * mb
            nc.sync.dma_start_transpose(
                out=tA[bi][:, c0:c0 + 4, 0:D], in_=rpb[0:D, :])
            nc.sync.dma_start_transpose(
                out=tA[bi][:, c0:c0 + 4, D:DH], in_=rpb[0:D, :])
            nc.sync.dma_start_transpose(
                out=tB[bi][:, c0:c0 + 4, 0:D], in_=rpb[D:DH, :])
            nc.sync.dma_start_transpose(
                out=tB[bi][:, c0:c0 + 4, D:DH], in_=rpb[D:DH, :])

            # ---- per-batch tail: DMA gathers + dots ----
            if mb == NMB - 1:
                idxt = gth.tile([128, 4, S // 16], i16, tag="idxs")
                nc.gpsimd.dma_start(out=idxt[:],
                                    in_=idx16[bi].rearrange("g p s -> p g s"))
                msk1 = gth.tile([1, S], f32, tag="msk1")
                nc.gpsimd.dma_start(out=msk1[:], in_=msk[bi:bi + 1, :])

                if gmode.startswith("hbm"):
                    # flush token-major tables to HBM for the gather
                    nc.sync.dma_start(
                        out=tAh[bi][:].rearrange("(c p) f -> p c f", p=128),
                        in_=tA[bi][:])
                    nc.sync.dma_start(
                        out=tBh[bi][:].rearrange("(c p) f -> p c f", p=128),
                        in_=tB[bi][:])
                    srcs = [tAh[bi], tAh[bi], tBh[bi], tBh[bi]]
                else:
                    srcs = [tA[bi], tA[bi], tB[bi], tB[bi]]

                # split each gather into GC-index chunks: the SWDGE ring
                # holds 128 in-flight descriptors/queue; a 2048-idx gather
                # needs 130 (> ring) and wedges the device. 1024 -> 66.
                GC = 1024
                g4 = []
                for j, tbl in enumerate(srcs):
                    if gmode == "hbm_nt":
                        gj = gth.tile([128, CPB, DH], bf16, tag=f"g{j}")
                    else:
                        gj = gth.tile([128, 1, S], bf16, tag=f"g{j}")
                    for h in range(S // GC):
                        isl = idxt[:, j, GC // 16 * h:GC // 16 * (h + 1)]
                        if gmode == "none":
                            continue
                        elif gmode == "hbm_t":
                            nc.gpsimd.dma_gather(
                                out_ap=gj[:, :, GC * h:GC * (h + 1)],
                                in_ap=tbl[:], idxs_ap=isl,
                                num_idxs=GC, num_idxs_reg=GC, elem_size=DH,
                                transpose=True)
                        elif gmode == "hbm_nt":
                            nc.gpsimd.dma_gather(
                                out_ap=gj[:, GC // 128 * h:GC // 128 * (h + 1), :],
                                in_ap=tbl[:], idxs_ap=isl,
                                num_idxs=GC, num_idxs_reg=GC, elem_size=DH,
                                transpose=False)
                        else:  # sbuf
                            nc.gpsimd.dma_gather(
                                out_ap=gj[:, :, GC * h:GC * (h + 1)],
                                in_ap=tbl[:], idxs_ap=isl,
                                num_idxs=GC, num_idxs_reg=GC, elem_size=DH,
                                transpose=True,
                                sbuf_tokens_per_rank=128,
                                sbuf_free_dim_per_rank=2 * DH,
                                sbuf_free_dim_pad_per_rank=0,
                                sbuf_byte_offset=0)
                    if gmode == "none":
                        nc.vector.memset(gj[:], 0.0)
                    g4.append(gj)

                if gmode == "hbm_nt":
                    # wrapped layout: relation r at [r%128, r//128, f].
                    # padd[.,.,0:64] = q(i0)k(i2), [.,.,64:] = q(i1)k(i3)
                    padd = gth.tile([128, CPB, DH], bf16, tag="prod")
                    nc.vector.tensor_tensor(out=padd[:, :, 0:D],
                                            in0=g4[0][:, :, 0:D],
                                            in1=g4[2][:, :, D:DH], op=Alu.mult)
                    nc.vector.tensor_tensor(out=padd[:, :, D:DH],
                                            in0=g4[1][:, :, 0:D],
                                            in1=g4[3][:, :, D:DH], op=Alu.mult)
                    # xbar: prodb[f, w, p] with free pos (c*128+p) = r
                    prodb = gth.tile([128, CPB, 128], bf16, tag="prodt")
                    nc.sync.dma_start_transpose(out=prodb[:], in_=padd[:])
                    red = prodb[:].rearrange("f c p -> f (c p)")
                else:
                    # feature-major gathers: prod[0:64] = q(i0)*k(i2),
                    # prod[64:] = q(i1)*k(i3)
                    prodb = gth.tile([DH, S], bf16, tag="prod")
                    nc.vector.tensor_tensor(out=prodb[0:D, :],
                                            in0=g4[0][0:D, 0, :],
                                            in1=g4[2][0:D, 0, :], op=Alu.mult)
                    nc.vector.tensor_tensor(out=prodb[D:DH, :],
                                            in0=g4[1][D:DH, 0, :],
                                            in1=g4[3][D:DH, 0, :], op=Alu.mult)
                    red = prodb[:]

                nc.scalar.activation(out=msk1[:], in_=msk1[:], func=Act.Copy,
                                     scale=1.25e11, bias=-1.25e11)
                lgsb = gth.tile([1, S], f32, tag="lgsb")
                for c in range(S // MT):
                    pl = ps_lg.tile([1, MT], f32, tag="lg")
                    nc.tensor.matmul(out=pl[:], lhsT=ones[:],
                                     rhs=red[:, MT * c:MT * (c + 1)],
                                     start=True, stop=True)
                    # fused /sqrt(D): logits * 0.125
                    nc.scalar.activation(out=lgsb[:, MT * c:MT * (c + 1)],
                                         in_=pl[:], func=Act.Copy, scale=0.125)
                nc.vector.tensor_tensor(out=lgsb[:], in0=lgsb[:], in1=msk1[:],
                                        op=Alu.add)
                nc.scalar.dma_start(out=out[bi:bi + 1, :], in_=lgsb[:])

    nc.compile()
    return nc


def _get_nc():
    if "nc" not in _CACHE:
        _CACHE["nc"] = _build_nc()
    return _CACHE["nc"]


def _shard(inputs):
    import ml_dtypes
    bf16 = ml_dtypes.bfloat16
    lhs = np.asarray(inputs["last_hidden_state"], np.float32)
    w = np.asarray(inputs["W"], np.float32)
    b = np.ascontiguousarray(np.asarray(inputs["b"], np.float32))
    pos = np.asarray(inputs["position_ids"], np.int32)
    rel = np.asarray(inputs["relations_idx"], np.int32)
    msk = np.asarray(inputs["labels_mask"], np.float32)

    wtb = np.ascontiguousarray(w.astype(bf16).reshape(KH, 128, DH))
    invf = np.power(10000.0, -np.arange(D // 2, dtype=np.float64) / (D / 2.0))

    in_maps = []
    for c in range(NCORES):
        sl = slice(BC * c, BC * (c + 1))
        # x^T bf16 tiles: xt[m, p, k, j] = x[512m+j, 128k+p]
        xc = lhs[sl].reshape(T, H).astype(bf16)
        xtc = np.ascontiguousarray(
            xc.T.reshape(KH, 128, NM, MT).transpose(2, 1, 0, 3))
        # feature-major sin/cos (interleaved pairs, tiled q|k)
        ang = pos[sl].astype(np.float64)[:, :, None] * invf  # [BC,S,32]
        sin2 = np.repeat(np.sin(ang), 2, axis=-1)            # [BC,S,64]
        cos2 = np.repeat(np.cos(ang), 2, axis=-1)
        sfm = np.concatenate([sin2, sin2], axis=-1).transpose(2, 0, 1)
        cfm = np.concatenate([cos2, cos2], axis=-1).transpose(2, 0, 1)
        sctc = np.ascontiguousarray(
            np.stack([sfm.reshape(DH, T), cfm.reshape(DH, T)]).astype(bf16))
        # wrapped int16 indices for dma_gather: idx[i] at [i%16, i//16],
        # replicated to 128 partitions (8 Q7 cores)
        wrp = (rel[sl].transpose(0, 2, 1)            # [BC, 4, S]
               .reshape(BC, 4, S // 16, 16)          # [BC, 4, s, j]
               .transpose(0, 1, 3, 2))               # [BC, 4, 16, s]
        idx16c = np.ascontiguousarray(
            np.tile(wrp, (1, 1, 8, 1)).astype(np.int16))
        in_maps.append({
            "xt": xtc,
            "wt": wtb,
            "bvec": b,
            "sct": sctc,
            "idx16": idx16c,
            "msk": np.ascontiguousarray(msk[sl]),
        })
    return in_maps


def kernel(**inputs):
    from concourse import bass_utils
    nc = _get_nc()
    in_maps = _shard(inputs)
    res = bass_utils.run_bass_kernel_spmd(
        nc, in_maps, core_ids=list(range(NCORES)))
    _CACHE["last_results"] = res
    outs = [res.results[c]["out"].reshape(BC, S) for c in range(NCORES)]
    return np.concatenate(outs, axis=0).astype(np.float32)


# revision 16
# speedup vs baseline: 3.0040x; 1.9653x over previous
"""Trainium2 Bass kernel for nn_CustomRelation (sparse_attention).

Per batch b:
    qkw = hidden @ W + bias            # [S, 128] = [q(64) | k(64)]
    RoPE(qkw) (interleaved pairs)
    logits[r] = q[i0[r]].k[i2[r]] + q[i1[r]].k[i3[r]]
    out = (logits + (1 - mask) * -1e12) / 8

Distribution: data-parallel over batch, 2 batches per NeuronCore x 8 cores.

Per-core dataflow (v2 — DMA-gather, host-prepped layouts):
  - Host pre-transposes x to bf16 x^T tiles [m, 128h, k, 512t] and
    precomputes feature-major sin/cos tables (bf16), so the device does
    no f32 x loads, no casts, no x transposes, and no sin/cos chain.
  - qkw^T [128f, 512t] = sum_k W_k^T @ XT_k on PE (bf16), +bias fused
    into the psum->sbuf bf16 cast on Scalar.
  - rot(qkw) via signed pair-swap permutation matmul (bf16).
  - rope = qkw*cos + rot*sin (DVE), output bf16 [128f, 512t].
  - One DMA-xbar transpose per tile packs rope token-major into a
    per-batch SBUF table: token t -> partition t%128, 256B row t//128.
  - 4x gpsimd.dma_gather (SWDGE, transpose mode, SBUF source) gather
    rows i0/i1/i2/i3 feature-major [128f, 2048r] in ~2-3us each
    (vs ~50us for the GPSIMD ap_gather ucode).
  - dots: prod[0:64] = G0.q * G2.k, prod[64:128] = G1.q * G3.k
    (partition-shifted DVE reads), ones-matmul partition reduction,
    mask+scale on [1, 2048]; store.
"""

import numpy as np

B, S, H, D = 16, 2048, 1024, 64
NCORES = 8
BC = B // NCORES            # batches per core
T = BC * S                  # tokens per core
DH = 2 * D                  # projected features (q|k)
MT = 512                    # macro-tile tokens
NM = T // MT                # macro-tiles per core
NMB = NM // BC              # macro-tiles per batch
KH = H // 128               # contraction chunks
CPB = S // 128              # 256B table rows per partition per batch

_CACHE = {}


def _build_nc():
    import concourse.bass as bass
    import concourse.tile as tile
    from concourse import bacc, mybir, library_config

    f32 = mybir.dt.float32
    bf16 = mybir.dt.bfloat16
    i16 = mybir.dt.int16
    Alu = mybir.AluOpType
    Act = mybir.ActivationFunctionType

    nc = bacc.Bacc("TRN2", target_bir_lowering=False, debug=False,
                   num_devices=NCORES, num_swdge_queues=4)

    xt = nc.dram_tensor("xt", [NM, 128, KH, MT], bf16, kind="ExternalInput")
    wt = nc.dram_tensor("wt", [KH, 128, DH], bf16, kind="ExternalInput")
    bvec = nc.dram_tensor("bvec", [DH], f32, kind="ExternalInput")
    sct = nc.dram_tensor("sct", [2, DH, T], bf16, kind="ExternalInput")
    idx16 = nc.dram_tensor("idx16", [BC, 4, 128, S // 16], i16,
                           kind="ExternalInput")
    msk = nc.dram_tensor("msk", [BC, S], f32, kind="ExternalInput")
    out = nc.dram_tensor("out", [BC, S], f32, kind="ExternalOutput")

    # --- constants baked into the NEFF ---
    import ml_dtypes
    pswapT = np.zeros((DH, DH), dtype=np.float32)       # signed pair swap
    for j in range(D):
        pswapT[2 * j + 1, 2 * j] = -1.0                 # rot[2j]   = -x[2j+1]
        pswapT[2 * j, 2 * j + 1] = 1.0                  # rot[2j+1] =  x[2j]
    pswap_t = nc.inline_tensor(pswapT.astype(ml_dtypes.bfloat16), "pswapT")
    ones_t = nc.inline_tensor(
        np.ones((DH, 1), dtype=ml_dtypes.bfloat16), "ones")

    import os
    # hbm_nt is the only dma_gather mode this runtime supports (SBUF-source
    # and transpose=True modes wedge the device; see transcript).
    gmode = os.environ.get("KDBG_GATHER", "hbm_nt")

    with tile.TileContext(nc) as tc, \
         tc.tile_pool(name="consts", bufs=1) as consts, \
         tc.tile_pool(name="xp", bufs=3) as xp, \
         tc.tile_pool(name="sc", bufs=3) as scp, \
         tc.tile_pool(name="sb", bufs=3) as sbp, \
         tc.tile_pool(name="fm", bufs=1) as fmp, \
         tc.tile_pool(name="gth", bufs=2) as gth, \
         tc.tile_pool(name="tblh", bufs=1, space="DRAM") as tblh, \
         tc.tile_pool(name="ps_qkw", bufs=2, space="PSUM") as ps_qkw, \
         tc.tile_pool(name="ps_rot", bufs=2, space="PSUM") as ps_rot, \
         tc.tile_pool(name="ps_lg", bufs=2, space="PSUM") as ps_lg:

        nc.gpsimd.load_library(library_config.mlp)

        # ---- constants / small inputs ----
        pswap = consts.tile([DH, DH], bf16, tag="pswap")
        nc.sync.dma_start(out=pswap[:], in_=pswap_t.ap())
        ones = consts.tile([DH, 1], bf16, tag="ones")
        nc.sync.dma_start(out=ones[:], in_=ones_t.ap())
        bcol = consts.tile([DH, 1], f32, tag="bcol")
        nc.sync.dma_start(out=bcol[:], in_=bvec.ap().rearrange("(p o) -> p o", o=1))
        wsb = consts.tile([128, KH, DH], bf16, tag="wsb")
        nc.sync.dma_start(out=wsb[:], in_=wt.ap().rearrange("k p f -> p k f"))

        # per-batch token-major rope table (rows = [q|k]): token t ->
        # partition t%128, 256B row t//128; flushed to HBM for the gather.
        # One table suffices: in the wrapped gather layout features sit on
        # the FREE dim, where shifted q/k slices are legal DVE operands.
        tT = [fmp.tile([128, CPB, DH], bf16, name=f"tT{b}", tag=f"tT{b}")
              for b in range(BC)]
        tTh = [tblh.tile([S, DH], bf16, name=f"tTh{b}", tag=f"tTh{b}")
               for b in range(BC)]

        # ---- main pipeline over macro-tiles ----
        for m in range(NM):
            bi = m // NMB
            mb = m % NMB
            t0 = m * MT

            xtile = xp.tile([128, KH, MT], bf16, tag="x")
            nc.sync.dma_start(out=xtile[:], in_=xt[m])
            sins = scp.tile([DH, MT], bf16, tag="sin")
            nc.scalar.dma_start(out=sins[:], in_=sct[0, :, t0:t0 + MT])
            coss = scp.tile([DH, MT], bf16, tag="cos")
            nc.scalar.dma_start(out=coss[:], in_=sct[1, :, t0:t0 + MT])

            # qkw^T [128f, 512t] = sum_k W_k^T @ XT_k
            pq = ps_qkw.tile([DH, MT], f32, tag="qkw")
            for k in range(KH):
                nc.tensor.matmul(out=pq[:], lhsT=wsb[:, k, :],
                                 rhs=xtile[:, k, :],
                                 start=(k == 0), stop=(k == KH - 1))
            qkwb = sbp.tile([DH, MT], bf16, tag="qkwb")
            nc.scalar.activation(out=qkwb[:], in_=pq[:], func=Act.Identity,
                                 bias=bcol[:])

            # rot(qkw)
            pr = ps_rot.tile([DH, MT], f32, tag="rot")
            nc.tensor.matmul(out=pr[:], lhsT=pswap[:], rhs=qkwb[:],
                             start=True, stop=True)

            # rope = qkw*cos + rot*sin  -> bf16
            t1 = sbp.tile([DH, MT], f32, tag="t1")
            nc.vector.tensor_tensor(out=t1[:], in0=qkwb[:], in1=coss[:],
                                    op=Alu.mult)
            t2 = sbp.tile([DH, MT], f32, tag="t2")
            nc.vector.tensor_tensor(out=t2[:], in0=pr[:], in1=sins[:],
                                    op=Alu.mult)
            rpb = sbp.tile([DH, MT], bf16, tag="rpb")
            nc.vector.tensor_tensor(out=rpb[:], in0=t1[:], in1=t2[:],
                                    op=Alu.add)

            # pack token-major into the per-batch table (xbar transpose):
            # token t=128c+p of this tile -> tT[p, 4*mb+c, :]
            c0 = 4 * mb
            nc.sync.dma_start_transpose(
                out=tT[bi][:, c0:c0 + 4, :], in_=rpb[:])

            # ---- per-batch tail: DMA gathers + dots ----
            if mb == NMB - 1:
                idxt = gth.tile([128, 4, S // 16], i16, tag="idxs")
                nc.sync.dma_start(out=idxt[:],
                                  in_=idx16[bi].rearrange("g p s -> p g s"))
                msk1 = gth.tile([1, S], f32, tag="msk1")
                nc.scalar.dma_start(out=msk1[:], in_=msk[bi:bi + 1, :])

                # flush the token-major table to HBM for the gather
                nc.sync.dma_start(
                    out=tTh[bi][:].rearrange("(c p) f -> p c f", p=128),
                    in_=tT[bi][:])

                # split each gather into GC-index chunks: the SWDGE ring
                # holds 128 in-flight descriptors/queue; a 2048-idx gather
                # needs 130 (> ring) and wedges the device. 1024 -> 66.
                # Rotate SWDGE queues so ring drains overlap desc-gen.
                GC = 1024
                g4 = []
                for j in range(4):
                    gj = gth.tile([128, CPB, DH], bf16, tag=f"g{j}")
                    for h in range(S // GC):
                        isl = idxt[:, j, GC // 16 * h:GC // 16 * (h + 1)]
                        if gmode == "none":
                            continue
                        nc.gpsimd.dma_gather(
                            out_ap=gj[:, GC // 128 * h:GC // 128 * (h + 1), :],
                            in_ap=tTh[bi][:], idxs_ap=isl,
                            num_idxs=GC, num_idxs_reg=GC, elem_size=DH,
                            transpose=False,
                            queue_num=(2 * j + h) % 4)
                    if gmode == "none":
                        nc.vector.memset(gj[:], 0.0)
                    g4.append(gj)

                # wrapped layout: relation r at [r%128, r//128, f].
                # padd[.,.,0:64] = q(i0)k(i2), [.,.,64:] = q(i1)k(i3)
                # (q/k feature halves are FREE-dim slices of the [q|k] rows)
                padd = gth.tile([128, CPB, DH], bf16, tag="prod")
                nc.vector.tensor_tensor(out=padd[:, :, 0:D],
                                        in0=g4[0][:, :, 0:D],
                                        in1=g4[2][:, :, D:DH], op=Alu.mult)
                nc.vector.tensor_tensor(out=padd[:, :, D:DH],
                                        in0=g4[1][:, :, 0:D],
                                        in1=g4[3][:, :, D:DH], op=Alu.mult)
                # xbar: prodb[f, w, p] with free pos (c*128+p) = r
                prodb = gth.tile([128, CPB, 128], bf16, tag="prodt")
                nc.sync.dma_start_transpose(out=prodb[:], in_=padd[:])
                red = prodb[:].rearrange("f c p -> f (c p)")

                nc.scalar.activation(out=msk1[:], in_=msk1[:], func=Act.Copy,
                                     scale=1.25e11, bias=-1.25e11)
                lgsb = gth.tile([1, S], f32, tag="lgsb")
                for c in range(S // MT):
                    pl = ps_lg.tile([1, MT], f32, tag="lg")
                    nc.tensor.matmul(out=pl[:], lhsT=ones[:],
                                     rhs=red[:, MT * c:MT * (c + 1)],
                                     start=True, stop=True)
                    # fused /sqrt(D): logits * 0.125
                    nc.scalar.activation(out=lgsb[:, MT * c:MT * (c + 1)],
                                         in_=pl[:], func=Act.Copy, scale=0.125)
                nc.vector.tensor_tensor(out=lgsb[:], in0=lgsb[:], in1=msk1[:],
                                        op=Alu.add)
                nc.scalar.dma_start(out=out[bi:bi + 1, :], in_=lgsb[:])

    nc.compile()
    return nc


def _get_nc():
    if "nc" not in _CACHE:
        _CACHE["nc"] = _build_nc()
    return _CACHE["nc"]


def _shard(inputs):
    import ml_dtypes
    bf16 = ml_dtypes.bfloat16
    lhs = np.asarray(inputs["last_hidden_state"], np.float32)
    w = np.asarray(inputs["W"], np.float32)
    b = np.ascontiguousarray(np.asarray(inputs["b"], np.float32))
    pos = np.asarray(inputs["position_ids"], np.int32)
    rel = np.asarray(inputs["relations_idx"], np.int32)
    msk = np.asarray(inputs["labels_mask"], np.float32)

    wtb = np.ascontiguousarray(w.astype(bf16).reshape(KH, 128, DH))
    invf = np.power(10000.0, -np.arange(D // 2, dtype=np.float64) / (D / 2.0))

    in_maps = []
    for c in range(NCORES):
        sl = slice(BC * c, BC * (c + 1))
        # x^T bf16 tiles: xt[m, p, k, j] = x[512m+j, 128k+p]
        xc = lhs[sl].reshape(T, H).astype(bf16)
        xtc = np.ascontiguousarray(
            xc.T.reshape(KH, 128, NM, MT).transpose(2, 1, 0, 3))
        # feature-major sin/cos (interleaved pairs, tiled q|k)
        ang = pos[sl].astype(np.float64)[:, :, None] * invf  # [BC,S,32]
        sin2 = np.repeat(np.sin(ang), 2, axis=-1)            # [BC,S,64]
        cos2 = np.repeat(np.cos(ang), 2, axis=-1)
        sfm = np.concatenate([sin2, sin2], axis=-1).transpose(2, 0, 1)
        cfm = np.concatenate([cos2, cos2], axis=-1).transpose(2, 0, 1)
        sctc = np.ascontiguousarray(
            np.stack([sfm.reshape(DH, T), cfm.reshape(DH, T)]).astype(bf16))
        # wrapped int16 indices for dma_gather: idx[i] at [i%16, i//16],
        # replicated to 128 partitions (8 Q7 cores)
        wrp = (rel[sl].transpose(0, 2, 1)            # [BC, 4, S]
               .reshape(BC, 4, S // 16, 16)          # [BC, 4, s, j]
               .transpose(0, 1, 3, 2))               # [BC, 4, 16, s]
        idx16c = np.ascontiguousarray(
            np.tile(wrp, (1, 1, 8, 1)).astype(np.int16))
        in_maps.append({
            "xt": xtc,
            "wt": wtb,
            "bvec": b,
            "sct": sctc,
            "idx16": idx16c,
            "msk": np.ascontiguousarray(msk[sl]),
        })
    return in_maps


def kernel(**inputs):
    from concourse import bass_utils
    nc = _get_nc()
    in_maps = _shard(inputs)
    res = bass_utils.run_bass_kernel_spmd(
        nc, in_maps, core_ids=list(range(NCORES)))
    _CACHE["last_results"] = res
    outs = [res.results[c]["out"].reshape(BC, S) for c in range(NCORES)]
    return np.concatenate(outs, axis=0).astype(np.float32)
